# revision 22
# baseline (speedup 1.0000x reference)
"""Trainium2 Bass kernel for channel attention (XCA-style) over 8 NeuronCores.

Module: q/k/v = depthwise3x3(1x1conv(x)); L2-normalize q,k over spatial;
channel attention per head (softmax over d=64 channel dim); 1x1 proj.

Sharding: data-parallel over batch B=8 -> 1 batch item per core, no
collectives.  All shapes hardcoded; weights pre-transposed on host.

Key structure (per core, per 128-channel chunk):
- 1x1 convs: bf16 PE matmuls, channel-major [c, s].
- depthwise 3x3: 9 taps over a zero-bordered [66, 66] padded copy of the
  pw output; per-chunk either on PE (9 accumulating diag-matmuls per PSUM
  bank) or on DVE (scalar_tensor_tensor FMA chain) -- DW_PE_CHUNKS knob.
- L2 norm: ACT Square+accum_out, Newton-refined rsqrt; q's 1/|q| folds
  into the softmax exp scale, k's into a cheap DVE row-scale.
- q/k transposed to [n, c] via DMA xbar transpose (free on compute
  engines); softmax 1/sum folds into the attnT-building PE matmul via
  diag(1/sum); attn@v and proj are plain PE matmuls.
"""

import os
import sys

import numpy as np

for _p in ("/opt/trn_rl_repo", "/root/.axon_site/_ro/trn_rl_repo"):
    if os.path.isdir(_p) and _p not in sys.path:
        sys.path.insert(0, _p)

import ml_dtypes

B, C, HH, WW = 8, 512, 64, 64
HEADS, D = 8, 64
HW = HH * WW          # 4096
G = C // 128          # 4 channel chunks of 128
NBK = 512             # matmul N (one PSUM bank of fp32)
NB = HW // NBK        # 8
PP = WW + 2           # padded h/w (66)
EPS = 1e-12

# which (tensor, chunk) run their depthwise on PE (else DVE)
DW_PE_CHUNKS = {("v", 0), ("v", 1), ("v", 2), ("v", 3),
                ("q", 0), ("q", 2), ("k", 0), ("k", 2)}

_CACHE = {}


def _build():
    """Build the single-core Bass program (SPMD: same program, per-core data)."""
    from contextlib import ExitStack

    import concourse.bass as bass
    import concourse.tile as tile
    from concourse import bacc, mybir

    f32 = mybir.dt.float32
    bf16 = mybir.dt.bfloat16
    AO = mybir.AluOpType
    AF = mybir.ActivationFunctionType

    # Bacc (not raw Bass): its compile() moves excess matmul waits onto
    # ldweights and splits >1-wait instructions onto InstEventSemaphore —
    # TRN2 allows only 1 sync wait per instruction.
    nc = bacc.Bacc()

    x_ext = nc.declare_dram_parameter("x", [C, HW], bf16, isOutput=False)
    w_ext = {
        t: nc.declare_dram_parameter(f"w{t}", [C, C], bf16, isOutput=False)
        for t in "qkv"
    }
    wp_ext = nc.declare_dram_parameter("wp", [C, C], bf16, isOutput=False)
    dw_ext = {
        t: nc.declare_dram_parameter(f"dw{t}", [C, 9], f32, isOutput=False)
        for t in "qkv"
    }
    tsc_ext = nc.declare_dram_parameter("tsc", [C, 1], f32, isOutput=False)
    id_ext = nc.declare_dram_parameter("ident", [128, 128], bf16, isOutput=False)
    out_ext = nc.declare_dram_parameter("out", [C, HW], f32, isOutput=True)

    with ExitStack() as ctx:
        tc = ctx.enter_context(tile.TileContext(nc))
        sb = ctx.enter_context(tc.tile_pool(name="sb", bufs=1))
        ps = ctx.enter_context(tc.tile_pool(name="ps", bufs=1, space="PSUM"))

        # ---- persistent loads -------------------------------------------
        # first pw matmul needs wq[k] + x[k]: load those first, interleaved
        x_sb = [None] * G
        w_sb = {t: [None] * G for t in "qkv"}
        for k in range(G):
            wt = sb.tile([128, C], bf16, name=f"wq{k}", tag=f"wq{k}")
            nc.sync.dma_start(out=wt, in_=w_ext["q"][k * 128:(k + 1) * 128, :])
            w_sb["q"][k] = wt
            xg = sb.tile([128, HW], bf16, name=f"x{k}", tag=f"x{k}")
            nc.sync.dma_start(out=xg, in_=x_ext[k * 128:(k + 1) * 128, :])
            x_sb[k] = xg
        for t in "kv":
            for k in range(G):
                wt = sb.tile([128, C], bf16, name=f"w{t}{k}", tag=f"w{t}{k}")
                nc.sync.dma_start(out=wt, in_=w_ext[t][k * 128:(k + 1) * 128, :])
                w_sb[t][k] = wt
        wp_sb = []  # loaded later into dead wq slots

        dw_sb = {}
        for t in "qkv":
            dw_sb[t] = []
            for g in range(G):
                d = sb.tile([128, 9], f32, name=f"dw{t}{g}", tag=f"dw{t}{g}")
                nc.sync.dma_start(out=d, in_=dw_ext[t][g * 128:(g + 1) * 128, :])
                dw_sb[t].append(d)

        tsc_sb = []
        for g in range(G):
            tg = sb.tile([128, 1], f32, name=f"tsc{g}", tag=f"tsc{g}")
            nc.sync.dma_start(out=tg, in_=tsc_ext[g * 128:(g + 1) * 128, :])
            tsc_sb.append(tg)

        ident = sb.tile([128, 128], bf16, name="ident", tag="ident")
        nc.sync.dma_start(out=ident, in_=id_ext[:, :])

        # persistent per-chunk results
        vdw = [sb.tile([128, HW], bf16, name=f"vdw{g}", tag=f"vdw{g}")
               for g in range(G)]
        qT = [sb.tile([128, HW], bf16, name=f"qT{g}", tag=f"qT{g}")
              for g in range(G)]
        kT = [sb.tile([128, HW], bf16, name=f"kT{g}", tag=f"kT{g}")
              for g in range(G)]
        # per-pair softmax-scale (T / |q|) vectors
        ts_scale = [sb.tile([128, 1], f32, name=f"tss{g}", tag=f"tss{g}")
                    for g in range(G)]

        # ---- pw conv + depthwise for one (tensor, chunk) ----------------
        def pw_dw(t, g, acc):
            """1x1 conv chunk g of tensor t into a zero-bordered [66,66]
            padded SBUF tile, then 3x3 depthwise into acc [128, HW] bf16.
            Every tap reads a full 64x64 window of the padded tile
            (borders supply the conv zeros)."""
            pad = sb.tile([128, PP, PP], bf16, name=f"pad_{t}{g}", tag="pwpad",
                          bufs=3)
            padf = pad.rearrange("p h w -> p (h w)")
            # zero the border: top/bottom rows + left/right cols
            nc.gpsimd.memset(padf[:, 0:PP], 0.0)
            nc.gpsimd.memset(padf[:, (PP - 1) * PP:PP * PP], 0.0)
            nc.gpsimd.memset(pad[:, 1:PP - 1, 0:1], 0.0)
            nc.gpsimd.memset(pad[:, 1:PP - 1, PP - 1:PP], 0.0)
            # 1x1 conv: bank-pair loop so each stationary w block serves
            # two matmuls (halves LDWEIGHTS traffic)
            for nb2 in range(NB // 2):
                pp0 = ps.tile([128, NBK], f32, name=f"pwa_{t}{g}{nb2}",
                              tag="ps_pw", bufs=2)
                pp1 = ps.tile([128, NBK], f32, name=f"pwb_{t}{g}{nb2}",
                              tag="ps_pw", bufs=2)
                for k in range(G):
                    lhs = w_sb[t][k][:, g * 128:(g + 1) * 128]
                    for j, pp in enumerate((pp0, pp1)):
                        nb = nb2 * 2 + j
                        nc.tensor.matmul(
                            pp, lhsT=lhs,
                            rhs=x_sb[k][:, nb * NBK:(nb + 1) * NBK],
                            start=(k == 0), stop=(k == G - 1),
                        )
                for j, pp in enumerate((pp0, pp1)):
                    nb = nb2 * 2 + j
                    eng = nc.scalar.copy if j == 0 else nc.vector.tensor_copy
                    eng(pad[:, 1 + nb * 8:1 + (nb + 1) * 8, 1:WW + 1],
                        pp.rearrange("p (h w) -> p h w", w=WW))

            acc3 = acc.rearrange("p (h w) -> p h w", w=WW)
            dwc = dw_sb[t][g]
            if (t, g) in DW_PE_CHUNKS:
                # PE path: per output bank (8 rows), 9 accumulating
                # diag-matmuls; diag(dw_tap) as stationary, padded window
                # as moving operand.
                diags = []
                for tap in range(9):
                    dg = sb.tile([128, 128], bf16, name=f"dg_{t}{g}{tap}",
                                 tag=f"dwdiag{tap}", bufs=1)
                    nc.vector.tensor_scalar(
                        out=dg, in0=ident, scalar1=dwc[:, tap:tap + 1],
                        scalar2=None, op0=AO.mult)
                    diags.append(dg)
                for nb2 in range(NB // 2):
                    dp0 = ps.tile([128, NBK], f32, name=f"dwa_{t}{g}{nb2}",
                                  tag="ps_dw", bufs=3)
                    dp1 = ps.tile([128, NBK], f32, name=f"dwb_{t}{g}{nb2}",
                                  tag="ps_dw", bufs=3)
                    for tap in range(9):
                        dy, dx = tap // 3, tap % 3
                        for j, dp in enumerate((dp0, dp1)):
                            r0 = (nb2 * 2 + j) * 8
                            nc.tensor.matmul(
                                dp, lhsT=diags[tap],
                                rhs=pad[:, dy + r0:dy + r0 + 8, dx:dx + WW],
                                start=(tap == 0), stop=(tap == 8),
                            )
                    for j, dp in enumerate((dp0, dp1)):
                        nb = nb2 * 2 + j
                        eng = (nc.scalar.copy if j == 0
                               else nc.vector.tensor_copy)
                        eng(acc[:, nb * NBK:(nb + 1) * NBK], dp)
                return pad
            else:
                # DVE path: center tap via 2x/4x tensor_scalar init, then
                # 8 in-place 1x scalar_tensor_tensor FMAs
                nc.vector.tensor_scalar(
                    out=acc3[:, :, :], in0=pad[:, 1:1 + HH, 1:1 + WW],
                    scalar1=dwc[:, 4:5], scalar2=None, op0=AO.mult)
                for tap in (0, 1, 2, 3, 5, 6, 7, 8):
                    dy, dx = tap // 3, tap % 3
                    nc.vector.scalar_tensor_tensor(
                        out=acc3[:, :, :],
                        in0=pad[:, dy:dy + HH, dx:dx + WW],
                        scalar=dwc[:, tap:tap + 1],
                        in1=acc3[:, :, :],
                        op0=AO.mult, op1=AO.add)
                return pad

        # ---- rsqrt of sum-of-squares along free dim ---------------------
        def rnorm(src, g, t, pad):
            """returns [128,1] f32 tile = 1/max(||src row||, eps).
            Squares scratch is written over the chunk's dead pad tile."""
            padf = pad.rearrange("p h w -> p (h w)")
            ss = sb.tile([128, 1], f32, name=f"ss_{t}{g}", tag="nrm_ss",
                         bufs=2)
            nc.scalar.activation(out=padf[:, 0:HW], in_=src, func=AF.Square,
                                 accum_out=ss)
            nc.vector.tensor_scalar(out=ss, in0=ss, scalar1=EPS * EPS,
                                    scalar2=None, op0=AO.max)
            sr = sb.tile([128, 1], f32, name=f"sr_{t}{g}", tag="nrm_sr",
                         bufs=2)
            nc.scalar.activation(out=sr, in_=ss, func=AF.Sqrt)
            r0_ = sb.tile([128, 1], f32, name=f"r0_{t}{g}", tag="nrm_r0",
                          bufs=2)
            nc.vector.reciprocal(r0_, sr)
            # one Newton step: r = r0*(1.5 - 0.5*ss*r0^2)  (ACT sqrt is loose)
            tn = sb.tile([128, 1], f32, name=f"tn_{t}{g}", tag="nrm_tn",
                         bufs=2)
            nc.vector.tensor_tensor(out=tn, in0=r0_, in1=r0_, op=AO.mult)
            nc.vector.tensor_tensor(out=tn, in0=tn, in1=ss, op=AO.mult)
            nc.vector.tensor_scalar(out=tn, in0=tn, scalar1=-0.5, scalar2=1.5,
                                    op0=AO.mult, op1=AO.add)
            rinv = sb.tile([128, 1], f32, name=f"ri_{t}{g}", tag=f"ri_{t}{g}")
            nc.vector.tensor_tensor(out=rinv, in0=r0_, in1=tn, op=AO.mult)
            return rinv

        # ---- attention for one head-pair (softmax folds into PE ops) ----
        o_sb = [None] * G

        def attn_pair(g):
            ap_ = ps.tile([128, 128], f32, name=f"attn{g}", tag="ps_attn",
                          bufs=1)
            for nck in range(32):
                nc.tensor.matmul(
                    ap_,
                    lhsT=qT[g][:, nck * 128:(nck + 1) * 128],
                    rhs=kT[g][:, nck * 128:(nck + 1) * 128],
                    start=(nck == 0), stop=(nck == 31))
            aexp = sb.tile([128, 128], bf16, name=f"aexp{g}", tag="aexp",
                           bufs=2)
            nc.vector.memset(aexp, 0.0)
            sume = sb.tile([128, 1], f32, name=f"sume{g}", tag="sume", bufs=2)
            for blk in (0, 64):
                nc.scalar.activation(
                    out=aexp[blk:blk + 64, blk:blk + 64],
                    in_=ap_[blk:blk + 64, blk:blk + 64],
                    func=AF.Exp, scale=ts_scale[g][blk:blk + 64, :],
                    accum_out=sume[blk:blk + 64, :])
            rs = sb.tile([128, 1], f32, name=f"rs{g}", tag="rsum", bufs=2)
            nc.vector.reciprocal(rs, sume)
            dgr = sb.tile([128, 128], bf16, name=f"dgr{g}", tag="diagr",
                          bufs=2)
            nc.vector.tensor_scalar(out=dgr, in0=ident, scalar1=rs,
                                    scalar2=None, op0=AO.mult)
            atp = ps.tile([128, 128], f32, name=f"atp{g}", tag="ps_vo",
                          bufs=2)
            nc.tensor.matmul(atp, lhsT=aexp, rhs=dgr, start=True, stop=True)
            attnT = sb.tile([128, 128], bf16, name=f"attnT{g}", tag="attnT",
                            bufs=2)
            nc.vector.tensor_copy(attnT, atp)
            og = sb.tile([128, HW], bf16, name=f"o{g}", tag=f"x{g}")
            for nb in range(NB):
                vo = ps.tile([128, NBK], f32, name=f"vo{g}{nb}", tag="ps_vo",
                             bufs=2)
                nc.tensor.matmul(vo, lhsT=attnT,
                                 rhs=vdw[g][:, nb * NBK:(nb + 1) * NBK],
                                 start=True, stop=True)
                nc.scalar.copy(og[:, nb * NBK:(nb + 1) * NBK], vo)
            o_sb[g] = og

        # ======= phases B+C: pw+dw for all (tensor, chunk), interleaved ====
        # order alternates DVE-dw chunks with PE-dw chunks so the PE always
        # has matmul work while a DVE FMA chain runs
        def post_qk(t, g, acc, pad):
            rinv = rnorm(acc, g, t, pad)
            if t == "q":
                # fold 1/|q| into the softmax exp scale; transpose raw q
                nc.vector.tensor_tensor(out=ts_scale[g], in0=tsc_sb[g],
                                        in1=rinv, op=AO.mult)
                src_ = acc
            else:
                # k-hat = k / |k| (DVE 4x), then transpose
                kh = sb.tile([128, HW], bf16, name=f"kh{g}", tag="dwork",
                             bufs=3)
                nc.vector.tensor_scalar(out=kh, in0=acc, scalar1=rinv,
                                        scalar2=None, op0=AO.mult)
                src_ = kh
            dstT = (qT if t == "q" else kT)[g]
            dst3 = dstT.rearrange("p (a c) -> p a c", c=128)
            nc.sync.dma_start(out=dst3, in_=src_, transpose=True)

        ORDER = [("q", 0), ("q", 1), ("v", 0), ("q", 2), ("q", 3),
                 ("v", 1), ("k", 0), ("A", 0), ("k", 1), ("A", 1),
                 ("v", 2), ("k", 2), ("A", 2), ("k", 3), ("v", 3),
                 ("A", 3)]
        for t, g in ORDER:
            if t == "A":
                attn_pair(g)
            elif t == "v":
                pw_dw("v", g, vdw[g])
            else:
                acc = sb.tile([128, HW], bf16, name=f"acc_{t}{g}", tag="dwork",
                              bufs=3)
                pad = pw_dw(t, g, acc)
                post_qk(t, g, acc, pad)
            if t == "q" and g == 3:
                # wq slots are dead after q's pw: load proj weights into them
                for k in range(G):
                    wpt = sb.tile([128, C], bf16, name=f"wp{k}",
                                  tag=f"wq{k}")
                    nc.sync.dma_start(out=wpt,
                                      in_=wp_ext[k * 128:(k + 1) * 128, :])
                    wp_sb.append(wpt)

        # ================= phase E: projection + store ===================
        for m in range(G):
            for nb in range(NB):
                # ps_vo tag: its previous reader is a DVE copy, keeping the
                # proj matmul fan-in small
                yp = ps.tile([128, NBK], f32, name=f"yp{m}{nb}", tag="ps_dw",
                             bufs=3)
                for g in range(G):
                    nc.tensor.matmul(
                        yp,
                        lhsT=wp_sb[g][:, m * 128:(m + 1) * 128],
                        rhs=o_sb[g][:, nb * NBK:(nb + 1) * NBK],
                        start=(g == 0), stop=(g == G - 1))
                yt = sb.tile([128, NBK], f32, name=f"yt{m}{nb}", tag="ysb",
                             bufs=2)
                nc.scalar.copy(yt, yp)
                nc.sync.dma_start(
                    out=out_ext[m * 128:(m + 1) * 128,
                                nb * NBK:(nb + 1) * NBK],
                    in_=yt)

    nc.compile()
    return nc


def _prep_inputs(x, w_q, w_k, w_v, dw_q, dw_k, dw_v, w_proj, temperature):
    bf16 = ml_dtypes.bfloat16
    xb = np.ascontiguousarray(np.asarray(x, np.float32)).reshape(B, C, HW)
    base = {
        "wq": np.ascontiguousarray(np.asarray(w_q, np.float32).T).astype(bf16),
        "wk": np.ascontiguousarray(np.asarray(w_k, np.float32).T).astype(bf16),
        "wv": np.ascontiguousarray(np.asarray(w_v, np.float32).T).astype(bf16),
        "wp": np.ascontiguousarray(np.asarray(w_proj, np.float32).T).astype(bf16),
        "dwq": np.asarray(dw_q, np.float32).reshape(C, 9).copy(),
        "dwk": np.asarray(dw_k, np.float32).reshape(C, 9).copy(),
        "dwv": np.asarray(dw_v, np.float32).reshape(C, 9).copy(),
        "tsc": np.repeat(np.asarray(temperature, np.float32).reshape(HEADS),
                         D).reshape(C, 1).copy(),
        "ident": np.eye(128, dtype=bf16),
    }
    in_maps = []
    for b in range(B):
        m = dict(base)
        m["x"] = xb[b].astype(bf16)
        in_maps.append(m)
    return in_maps


def run(trace=False, **inputs):
    from concourse.bass_utils import run_bass_kernel_spmd

    if "nc" not in _CACHE:
        _CACHE["nc"] = _build()
    nc = _CACHE["nc"]
    in_maps = _prep_inputs(**inputs)
    res = run_bass_kernel_spmd(nc, in_maps, core_ids=list(range(B)),
                               trace=trace)
    out = np.stack([np.asarray(res.results[b]["out"], np.float32)
                    for b in range(B)])
    return out.reshape(B, C, HH, WW), res


def kernel(**inputs):
    out, _ = run(trace=False, **inputs)
    return out
